# revision 31
# baseline (speedup 1.0000x reference)
"""Trainium2 Bass kernel for nn_DistancePredictor (pairwise MLP distance map).

out[b,i,j] = relu(W2 . gelu(cat(Xi,Xj,Xi-Xj,Xi*Xj) @ W1 + b1) + b2), symmetrized,
diagonal zeroed.  Decomposition (per row i):

    cat(...) @ W1 = X_j @ (Wp*X_i + (Wj-Wd)) + X_i @ (Wi+Wd)
                    `------- W_i (dxh) -----'   `--- A_i (bias) ---'

The row bias A_i + b1 is *seeded into PSUM by the PE* (a K=128 matmul of the
transposed-bias tile against a zero-stride broadcast identity column), so the
gelu needs no per-partition bias operand and one ACTIVATE can span 1.5 rows
(1536 cols = 3 PSUM banks).  That amortizes the ~185ns fixed ACT overhead per
instruction -- ACT is the bottleneck engine at ~94% busy.

Per 1536-col group: 3 bias-seed matmuls (fp16, start=True) + 3 fp32r S-matmuls
(accumulate, stop=True), one 1536-wide gelu (PSUM->SBUF fp16), then 12 128-col
x W2 matmuls that write the output *transposed* (j on partitions) into PSUM
accumulator banks.  Relu and the 0.5 symmetrize factor are folded into the
evacuation (W2,b2 pre-scaled by 0.5 on host).  The symmetrize term r'[j,i] is
fetched with a per-batch 8-core AllToAll of fp16 128x128 blocks (batch 0's
exchange overlaps batch 1's compute), transposed in-flight by the DMA xbar,
and added on GpSimd/DVE; the diagonal mask is per-core input data so the SPMD
program is identical on all cores.
"""

import numpy as np

import concourse.bacc as bacc
import concourse.mybir as mybir
import concourse.tile as tile
from concourse.bass_utils import run_bass_kernel_spmd

F32 = mybir.dt.float32
F32R = mybir.dt.float32r
F16 = mybir.dt.float16
AF = mybir.ActivationFunctionType
ALU = mybir.AluOpType

B, L, D = 2, 1024, 128
H = 128
NCORES = 8
SLAB = L // NCORES  # 128
NCHUNK = 2 * SLAB  # 512-col chunks per batch (2 per row)
NGROUP = (NCHUNK + 2) // 3  # 86: 85 full 1536-col groups + one 512-col tail


def build_nc(skip_collective=False, reps=1):
    nc = bacc.Bacc(
        "TRN2",
        target_bir_lowering=False,
        debug=False,
        num_devices=NCORES,
    )

    xt_in = nc.dram_tensor("xt", [B, D, L], F32R, kind="ExternalInput")
    xc_in = nc.dram_tensor("xc", [B, D, SLAB], F32, kind="ExternalInput")
    # packed constants: cst = [xc0 | wa | wp | wb | b1r | b2c] (fp32),
    # cst16 = [eye | w2h | masks] (fp16) -- one DMA each; a HWDGE queue
    # issues only ~1 DMA per 1.26us, so the startup must use few DMAs.
    cst_in = nc.dram_tensor("cst", [128, 641], F32, kind="ExternalInput")
    c16_in = nc.dram_tensor("cst16", [128, 1154], F16, kind="ExternalInput")
    out_t = nc.dram_tensor("out", [B, L, SLAB], F16, kind="ExternalOutput")

    with tile.TileContext(nc) as tc:
        with (
            tc.tile_pool(name="const", bufs=1) as cp,
            tc.tile_pool(name="wpool", bufs=6) as wp_pool,
            tc.tile_pool(name="gpool", bufs=4) as g_pool,
            tc.tile_pool(name="rt", bufs=1) as rt_pool,
            tc.tile_pool(name="fin", bufs=8) as fin_pool,
            tc.tile_pool(name="ps_s", bufs=1, space="PSUM") as ps_s,
            tc.tile_pool(name="ps_acc", bufs=1, space="PSUM") as ps_acc,
            tc.tile_pool(name="dram", bufs=1, space="DRAM") as dram_pool,
        ):
            # ---- load constants / inputs to SBUF; few DMAs per queue so
            # the first group's dependency chain resolves fast ----
            cst_sb = cp.tile([128, 641], F32, name="cst_sb")
            c16_sb = cp.tile([128, 1154], F16, name="c16_sb")
            xt_sb = [cp.tile([D, L], F32R, name=f"xt_sb{b}") for b in range(B)]
            xc1_sb = cp.tile([D, SLAB], F32, name="xc1_sb")

            xc_sb = [cst_sb[:, 0:128], xc1_sb[:]]
            wa_sb = cst_sb[:, 128:256]
            wp_sb = cst_sb[:, 256:384]
            wb_sb = cst_sb[:, 384:512]
            b1_sb = cst_sb[:, 512:640]
            b2_sb = cst_sb[:, 640:641]
            eye_sb = c16_sb[:, 0:128]
            w2_sb = c16_sb[:, 128:129]
            masks_sb = c16_sb[:, 130:1154]

            nc.sync.dma_start(cst_sb[:], cst_in[:])
            nc.sync.dma_start(xt_sb[0][:, 0:512], xt_in[0][:, 0:512])
            nc.sync.dma_start(xt_sb[0][:, 512:1024], xt_in[0][:, 512:1024])
            nc.scalar.dma_start(c16_sb[:], c16_in[:])
            nc.gpsimd.dma_start(xc1_sb[:], xc_in[1])
            nc.gpsimd.dma_start(xt_sb[1][:, 0:512], xt_in[1][:, 0:512])
            nc.gpsimd.dma_start(xt_sb[1][:, 512:1024], xt_in[1][:, 512:1024])

            # ---- PE p-state warm-up: the PE reaches full clock only after
            # ~3us of continuous activity, so spin zero matmuls (DVE-memset
            # input, dead PSUM output) from t~0 or the first real groups run
            # at half speed ----
            wz = cp.tile([128, 514], F16, name="wz")
            nc.vector.memset(wz[:], 0.0)
            acc_warm = ps_acc.tile([128, 512], F32, tag="acc", name="acc_warm")
            for _ in range(5):
                nc.tensor.matmul(
                    acc_warm[0:2, :], wz[:, 0:2], wz[:, 2:514],
                    start=True, stop=True,
                )

            # ---- atT[il, h] = Xc^T Wa + b1 (transposed bias, fp16) ----
            # Seeds read it as a K=128 stationary; the eye-column broadcast
            # picks the row.  Batch 1's chain is emitted only after batch 0's
            # first group: placed earlier, its DVE add gets scheduled between
            # the first wtile builds and deadlocks the PE<->DVE FIFOs for
            # ~3us (S-matmul waits wtile; wtile behind atT1-add; atT1-add
            # waits atT1-matmul; atT1-matmul behind S-matmul).
            at_sb = {}

            def emit_atT(b):
                at_ps = ps_s.tile(
                    [128, 1536], F32, tag="sA" if b == 0 else "sB", name=f"at_ps{b}"
                )
                nc.tensor.matmul(
                    at_ps[:, 0:H], xc_sb[b], wa_sb, start=True, stop=True
                )
                atb = cp.tile([SLAB, H], F16, name=f"at_sb{b}")
                nc.vector.tensor_tensor(
                    atb[:], at_ps[:, 0:H], b1_sb, op=ALU.add
                )
                at_sb[b] = atb

            emit_atT(0)

            # ---- A2A buffers in DRAM (per batch, so batch 0's exchange +
            # symmetrize overlap batch 1's compute) ----
            a2a_send = [
                dram_pool.tile([NCORES, SLAB, SLAB], F16, name=f"a2a_send{b}")
                for b in range(B)
            ]
            a2a_recv = [
                dram_pool.tile([NCORES, SLAB, SLAB], F16, name=f"a2a_recv{b}")
                for b in range(B)
            ]

            # ---- main loop ----
            # Chunks (512 cols each) are dealt to three rotating PSUM
            # buffers -- 1536/1536/512 cols -- so one gelu can span 1.5 rows
            # while the other two buffers refill; the W2 accumulators for
            # both j-halves share a single PSUM bank (il in 64-row phases,
            # evacuated twice per batch) to make the third buffer fit.
            rt_tiles = {}
            for rep, b in [(r, b) for r in range(reps) for b in range(B)]:
                xtr = xt_sb[b][:]
                wtiles = {}
                accs = None
                cur_phase = -1
                last_b = rep == reps - 1 and b == B - 1

                def emit_evac(ph, accs_, b=b, last_b=last_b):
                    if ph == 0:
                        rt_tiles[b] = rt_pool.tile(
                            [128, 8 * SLAB], F16, name=f"rt_{b}"
                        )
                    rt = rt_tiles[b]
                    # rt col layout (jt, il): jt*128 + il; acc col layout
                    # (q, sub, il-phase): q*256 + sub*64 + (il - 64*ph)
                    dst = rt[:].rearrange("r (s c) -> r s c", s=8)[
                        :, :, ph * 64 : (ph + 1) * 64
                    ]
                    src = accs_[0][:].rearrange("r (s c) -> r s c", s=8)
                    if last_b and ph == 1:
                        # ACT is idle after the final gelu; split the final
                        # evacuation so both halves go in parallel.
                        nc.vector.tensor_scalar(
                            dst[:, 0:4], src[:, 0:4], b2_sb, 0.0,
                            op0=ALU.add, op1=ALU.max,
                        )
                        nc.scalar.activation(
                            dst[:, 4:8], src[:, 4:8], AF.Relu, bias=b2_sb,
                            scale=1.0,
                        )
                    else:
                        nc.vector.tensor_scalar(
                            dst, src, b2_sb, 0.0, op0=ALU.add, op1=ALU.max
                        )

                groups = []
                c = 0
                cyc = [("sA", 3), ("sB", 3), ("sC", 1)]
                gi = 0
                while c < NCHUNK:
                    tag, n = cyc[gi % 3]
                    gi += 1
                    n = min(n, NCHUNK - c)
                    groups.append((tag, list(range(c, c + n))))
                    c += n

                for gidx, (tag, chunks) in enumerate(groups):
                    if b == 0 and gidx == 1 and 1 not in at_sb:
                        emit_atT(1)
                    glen = 512 * len(chunks)
                    width = 512 if tag == "sC" else 1536
                    ps = ps_s.tile([128, width], F32, tag=tag)
                    # bias seeds: ps[:, c] = atT[row] (broadcast identity col)
                    for ci, C in enumerate(chunks):
                        row = C // 2
                        nc.tensor.matmul(
                            ps[:, ci * 512 : (ci + 1) * 512],
                            at_sb[b][:],
                            eye_sb[:, row : row + 1].broadcast_to((128, 512)),
                            start=True,
                            stop=False,
                        )
                    # S accumulate: ps[:, c] += W_row^T @ X[:, cols]
                    for ci, C in enumerate(chunks):
                        row, xcol = C // 2, (C % 2) * 512
                        if row not in wtiles:
                            wt = wp_pool.tile([D, H], F32R, tag="wi")
                            nc.vector.scalar_tensor_tensor(
                                wt[:],
                                wp_sb,
                                xc_sb[b][:, row : row + 1],
                                wb_sb,
                                op0=ALU.mult,
                                op1=ALU.add,
                            )
                            wtiles = {row: wt}  # keep only the latest
                        nc.tensor.matmul(
                            ps[:, ci * 512 : (ci + 1) * 512],
                            wtiles[row][:],
                            xtr[:, xcol : xcol + 512],
                            start=False,
                            stop=True,
                        )
                    gt = g_pool.tile([128, 1536], F16, tag="g")
                    nc.scalar.activation(
                        gt[:, 0:glen], ps[:, 0:glen], AF.Gelu, bias=0.0, scale=1.0
                    )
                    for ci, C in enumerate(chunks):
                        il = C // 2
                        phase = il // 64
                        if phase != cur_phase:
                            if accs is not None:
                                emit_evac(cur_phase, accs)
                            accs = [
                                ps_acc.tile(
                                    [128, 512],
                                    F32,
                                    tag="acc",
                                    name=f"acc_{rep}_{b}_{phase}",
                                )
                            ]
                            cur_phase = phase
                        q = C % 2
                        for s in range(4):
                            sub = s  # jt % 4
                            col = q * 256 + sub * 64 + (il - 64 * phase)
                            nc.tensor.matmul(
                                accs[0][:, col : col + 1],
                                gt[:, (ci * 4 + s) * 128 : (ci * 4 + s + 1) * 128],
                                w2_sb,
                                start=True,
                                stop=True,
                            )
                emit_evac(cur_phase, accs)

                # mask + stage: zero this core's diagonal block BEFORE
                # staging so the values come back from the AllToAll already
                # masked; then one DMA to the A2A send buffer.
                rt = rt_tiles[b]
                # mask in halves so the first half's symmetrize adds can
                # start while the second evac half is still in flight
                nc.vector.tensor_tensor(
                    rt[:, 0:512], rt[:, 0:512], masks_sb[:, 0:512], op=ALU.mult
                )
                nc.vector.tensor_tensor(
                    rt[:, 512:1024], rt[:, 512:1024], masks_sb[:, 512:1024],
                    op=ALU.mult,
                )
                stage_eng = nc.gpsimd if last_b else nc.sync
                stage_eng.dma_start(
                    a2a_send[b][:].rearrange("s r c -> r s c"),
                    rt[:].rearrange("r (s c) -> r s c", s=8),
                )

                # all-to-all this batch's transposed-slab blocks
                if not skip_collective:
                    nc.gpsimd.collective_compute(
                        "AllToAll",
                        ALU.bypass,
                        replica_groups=[list(range(NCORES))],
                        ins=[a2a_send[b].opt()],
                        outs=[a2a_recv[b].opt()],
                    )


                # symmetrize: out[b, d-block, :] = own + recv^T (the diag
                # mask was already applied to rt pre-exchange).  One merged
                # output store; HWDGE issues only ~1 DMA per 1.26us, so the
                # tail must use few DMAs.
                ob_all = fin_pool.tile([128, 8 * SLAB], F16, name=f"ob_{b}")
                if last_b:
                    # final batch: one merged recv load, transposed on the
                    # (now idle) PE against the identity, adds on DVE.  Each
                    # transpose gets its own PSUM bank -- a shared bank would
                    # serialize transpose_d+1 behind add_d (bank-granular
                    # accumulation groups).
                    rv = fin_pool.tile([128, 8 * SLAB], F16, name=f"rv_{b}")
                    nc.scalar.dma_start(
                        rv[:].rearrange("r (d c) -> r d c", d=8),
                        a2a_recv[b][:].rearrange("d r c -> r d c"),
                    )
                    tb_sa = ps_s.tile([128, 3072], F16, tag="sA")
                    tb_sb = ps_s.tile([128, 3072], F16, tag="sB")
                    tb_sc = ps_s.tile([128, 1024], F16, tag="sC")
                    tb_ac = ps_acc.tile([128, 512], F32, tag="acc")
                    banks = (
                        [(tb_sa, 1024 * k) for k in range(3)]
                        + [(tb_sb, 1024 * k) for k in range(3)]
                        + [(tb_sc, 0), (tb_ac, 0)]
                    )
                    for d in range(NCORES):
                        sl = slice(d * 128, (d + 1) * 128)
                        tbt, off = banks[d]
                        dst = tbt[:, off : off + 256].bitcast(F16)[:, 0:128] if (
                            tbt is tb_ac
                        ) else tbt[:, off : off + 128]
                        nc.tensor.transpose(dst, rv[:, sl], eye_sb)
                        nc.vector.tensor_tensor(
                            ob_all[:, sl], dst, rt[:, sl], op=ALU.add
                        )
                        if d == 3:
                            nc.scalar.dma_start(
                                out_t[b, 0:512].rearrange("(d p) f -> p d f", d=4),
                                ob_all[:, 0:512].rearrange(
                                    "p (d f) -> p d f", d=4
                                ),
                            )
                    nc.sync.dma_start(
                        out_t[b, 512:1024].rearrange("(d p) f -> p d f", d=4),
                        ob_all[:, 512:1024].rearrange("p (d f) -> p d f", d=4),
                    )
                else:
                    # recv blocks transposed in-flight by the DMA xbar
                    for d in range(NCORES):
                        rbt = fin_pool.tile([128, 128], F16, tag="rbt")
                        nc.sync.dma_start_transpose(rbt[:], a2a_recv[b][d])
                        nc.gpsimd.tensor_tensor(
                            ob_all[:, d * 128 : (d + 1) * 128],
                            rbt[:],
                            rt[:, d * 128 : (d + 1) * 128],
                            op=ALU.add,
                        )
                if not last_b:
                    nc.gpsimd.dma_start(
                        out_t[b].rearrange("(d p) f -> p d f", d=8),
                        ob_all[:].rearrange("p (d f) -> p d f", d=8),
                    )

    nc.compile()
    return nc


_NC_CACHE = {}


def _get_nc():
    if "nc" not in _NC_CACHE:
        _NC_CACHE["nc"] = build_nc()
    return _NC_CACHE["nc"]


def make_in_maps(X, W1, b1, W2, b2):
    X = np.ascontiguousarray(X, dtype=np.float32)
    W1 = np.asarray(W1, dtype=np.float32)
    b1 = np.asarray(b1, dtype=np.float32)
    W2 = np.asarray(W2, dtype=np.float32)
    b2 = np.asarray(b2, dtype=np.float32)

    Wi, Wj, Wd, Wp = W1[0:128], W1[128:256], W1[256:384], W1[384:512]
    wa = Wi + Wd
    wb = Wj - Wd
    wp = Wp
    w2h = (0.5 * W2).astype(np.float16).reshape(H, 1)
    b1r = np.tile(b1.reshape(1, H), (128, 1))
    b2c = np.full((128, 1), 0.5 * float(b2[0]), dtype=np.float32)
    eye = np.eye(128, dtype=np.float16)
    xt = np.ascontiguousarray(X.transpose(0, 2, 1))  # (B, D, L)

    in_maps = []
    for c in range(NCORES):
        masks = np.ones((128, NCORES * 128), dtype=np.float16)
        masks[:, c * 128 : (c + 1) * 128] = (
            1.0 - np.eye(128)
        ).astype(np.float16)
        xc = np.ascontiguousarray(xt[:, :, c * SLAB : (c + 1) * SLAB])
        cst = np.ascontiguousarray(
            np.concatenate([xc[0], wa, wp, wb, b1r, b2c], axis=1)
        ).astype(np.float32)
        pad = np.zeros((128, 1), np.float16)
        cst16 = np.ascontiguousarray(
            np.concatenate([eye, w2h, pad, masks], axis=1)
        ).astype(np.float16)
        in_maps.append({"xt": xt, "xc": xc, "cst": cst, "cst16": cst16})
    return in_maps


def assemble(results):
    full = np.empty((B, L, L), dtype=np.float32)
    for c in range(NCORES):
        o = results[c]["out"]  # (B, L, SLAB) fp16: out[b, j, i_local]
        full[:, c * SLAB : (c + 1) * SLAB, :] = o.transpose(0, 2, 1).astype(
            np.float32
        )
    return full


def kernel(X, W1, b1, W2, b2, _trace=False):
    nc = _get_nc()
    in_maps = make_in_maps(X, W1, b1, W2, b2)
    res = run_bass_kernel_spmd(
        nc, in_maps, core_ids=list(range(NCORES)), trace=_trace
    )
    out = assemble(res.results)
    if _trace:
        return out, res
    return out


if __name__ == "__main__":
    nc = build_nc()
    print("compiled ok")


# revision 32
# speedup vs baseline: 1.0011x; 1.0011x over previous
"""Trainium2 Bass kernel for nn_DistancePredictor (pairwise MLP distance map).

out[b,i,j] = relu(W2 . gelu(cat(Xi,Xj,Xi-Xj,Xi*Xj) @ W1 + b1) + b2), symmetrized,
diagonal zeroed.  Decomposition (per row i):

    cat(...) @ W1 = X_j @ (Wp*X_i + (Wj-Wd)) + X_i @ (Wi+Wd)
                    `------- W_i (dxh) -----'   `--- A_i (bias) ---'

The row bias A_i + b1 is *seeded into PSUM by the PE* (a K=128 matmul of the
transposed-bias tile against a zero-stride broadcast identity column), so the
gelu needs no per-partition bias operand and one ACTIVATE can span 1.5 rows
(1536 cols = 3 PSUM banks).  That amortizes the ~185ns fixed ACT overhead per
instruction -- ACT is the bottleneck engine at ~94% busy.

Per 1536-col group: 3 bias-seed matmuls (fp16, start=True) + 3 fp32r S-matmuls
(accumulate, stop=True), one 1536-wide gelu (PSUM->SBUF fp16), then 12 128-col
x W2 matmuls that write the output *transposed* (j on partitions) into PSUM
accumulator banks.  Relu and the 0.5 symmetrize factor are folded into the
evacuation (W2,b2 pre-scaled by 0.5 on host).  The symmetrize term r'[j,i] is
fetched with a per-batch 8-core AllToAll of fp16 128x128 blocks (batch 0's
exchange overlaps batch 1's compute), transposed in-flight by the DMA xbar,
and added on GpSimd/DVE; the diagonal mask is per-core input data so the SPMD
program is identical on all cores.
"""

import numpy as np

import concourse.bacc as bacc
import concourse.mybir as mybir
import concourse.tile as tile
from concourse.bass_utils import run_bass_kernel_spmd

F32 = mybir.dt.float32
F32R = mybir.dt.float32r
F16 = mybir.dt.float16
AF = mybir.ActivationFunctionType
ALU = mybir.AluOpType

B, L, D = 2, 1024, 128
H = 128
NCORES = 8
SLAB = L // NCORES  # 128
NCHUNK = 2 * SLAB  # 512-col chunks per batch (2 per row)
NGROUP = (NCHUNK + 2) // 3  # 86: 85 full 1536-col groups + one 512-col tail


def build_nc(skip_collective=False, reps=1):
    nc = bacc.Bacc(
        "TRN2",
        target_bir_lowering=False,
        debug=False,
        num_devices=NCORES,
    )

    xt_in = nc.dram_tensor("xt", [B, D, L], F32R, kind="ExternalInput")
    xc_in = nc.dram_tensor("xc", [B, D, SLAB], F32, kind="ExternalInput")
    # packed constants: cst = [xc0 | wa | wp | wb | b1r | b2c] (fp32),
    # cst16 = [eye | w2h | masks] (fp16) -- one DMA each; a HWDGE queue
    # issues only ~1 DMA per 1.26us, so the startup must use few DMAs.
    cst_in = nc.dram_tensor("cst", [128, 641], F32, kind="ExternalInput")
    c16_in = nc.dram_tensor("cst16", [128, 1154], F16, kind="ExternalInput")
    out_t = nc.dram_tensor("out", [B, L, SLAB], F16, kind="ExternalOutput")

    with tile.TileContext(nc) as tc:
        with (
            tc.tile_pool(name="const", bufs=1) as cp,
            tc.tile_pool(name="wpool", bufs=6) as wp_pool,
            tc.tile_pool(name="gpool", bufs=4) as g_pool,
            tc.tile_pool(name="rt", bufs=1) as rt_pool,
            tc.tile_pool(name="fin", bufs=8) as fin_pool,
            tc.tile_pool(name="ps_s", bufs=1, space="PSUM") as ps_s,
            tc.tile_pool(name="ps_acc", bufs=1, space="PSUM") as ps_acc,
            tc.tile_pool(name="dram", bufs=1, space="DRAM") as dram_pool,
        ):
            # ---- load constants / inputs to SBUF; few DMAs per queue so
            # the first group's dependency chain resolves fast ----
            cst_sb = cp.tile([128, 641], F32, name="cst_sb")
            c16_sb = cp.tile([128, 1154], F16, name="c16_sb")
            xt_sb = [cp.tile([D, L], F32R, name=f"xt_sb{b}") for b in range(B)]
            xc1_sb = cp.tile([D, SLAB], F32, name="xc1_sb")

            xc_sb = [cst_sb[:, 0:128], xc1_sb[:]]
            wa_sb = cst_sb[:, 128:256]
            wp_sb = cst_sb[:, 256:384]
            wb_sb = cst_sb[:, 384:512]
            b1_sb = cst_sb[:, 512:640]
            b2_sb = cst_sb[:, 640:641]
            eye_sb = c16_sb[:, 0:128]
            w2_sb = c16_sb[:, 128:129]
            masks_sb = c16_sb[:, 130:1154]

            nc.sync.dma_start(cst_sb[:], cst_in[:])
            nc.sync.dma_start(xt_sb[0][:, 0:512], xt_in[0][:, 0:512])
            nc.sync.dma_start(xt_sb[0][:, 512:1024], xt_in[0][:, 512:1024])
            nc.scalar.dma_start(c16_sb[:], c16_in[:])
            nc.gpsimd.dma_start(xc1_sb[:], xc_in[1])
            nc.gpsimd.dma_start(xt_sb[1][:, 0:512], xt_in[1][:, 0:512])
            nc.gpsimd.dma_start(xt_sb[1][:, 512:1024], xt_in[1][:, 512:1024])

            # ---- PE p-state warm-up: the PE reaches full clock only after
            # ~3us of continuous activity, so spin zero matmuls (DVE-memset
            # input, dead PSUM output) from t~0 or the first real groups run
            # at half speed ----
            wz = cp.tile([128, 514], F16, name="wz")
            nc.vector.memset(wz[:], 0.0)
            acc_warm = ps_acc.tile([128, 512], F32, tag="acc", name="acc_warm")
            for _ in range(5):
                nc.tensor.matmul(
                    acc_warm[0:2, :], wz[:, 0:2], wz[:, 2:514],
                    start=True, stop=True,
                )

            # ---- atT[il, h] = Xc^T Wa + b1 (transposed bias, fp16) ----
            # Seeds read it as a K=128 stationary; the eye-column broadcast
            # picks the row.  Batch 1's chain is emitted only after batch 0's
            # first group: placed earlier, its DVE add gets scheduled between
            # the first wtile builds and deadlocks the PE<->DVE FIFOs for
            # ~3us (S-matmul waits wtile; wtile behind atT1-add; atT1-add
            # waits atT1-matmul; atT1-matmul behind S-matmul).
            at_sb = {}

            def emit_atT(b):
                at_ps = ps_s.tile(
                    [128, 1536], F32, tag="sA" if b == 0 else "sB", name=f"at_ps{b}"
                )
                nc.tensor.matmul(
                    at_ps[:, 0:H], xc_sb[b], wa_sb, start=True, stop=True
                )
                atb = cp.tile([SLAB, H], F16, name=f"at_sb{b}")
                nc.vector.tensor_tensor(
                    atb[:], at_ps[:, 0:H], b1_sb, op=ALU.add
                )
                at_sb[b] = atb

            emit_atT(0)

            # ---- A2A buffers in DRAM (per batch, so batch 0's exchange +
            # symmetrize overlap batch 1's compute) ----
            a2a_send = [
                dram_pool.tile([NCORES, SLAB, SLAB], F16, name=f"a2a_send{b}")
                for b in range(B)
            ]
            a2a_recv = [
                dram_pool.tile([NCORES, SLAB, SLAB], F16, name=f"a2a_recv{b}")
                for b in range(B)
            ]

            # ---- main loop ----
            # Chunks (512 cols each) are dealt to three rotating PSUM
            # buffers -- 1536/1536/512 cols -- so one gelu can span 1.5 rows
            # while the other two buffers refill; the W2 accumulators for
            # both j-halves share a single PSUM bank (il in 64-row phases,
            # evacuated twice per batch) to make the third buffer fit.
            rt_tiles = {}
            for rep, b in [(r, b) for r in range(reps) for b in range(B)]:
                xtr = xt_sb[b][:]
                wtiles = {}
                accs = None
                cur_phase = -1
                last_b = rep == reps - 1 and b == B - 1

                def emit_evac(ph, accs_, b=b, last_b=last_b):
                    if ph == 0:
                        rt_tiles[b] = rt_pool.tile(
                            [128, 8 * SLAB], F16, name=f"rt_{b}"
                        )
                    rt = rt_tiles[b]
                    # rt col layout (jt, il): jt*128 + il; acc col layout
                    # (q, sub, il-phase): q*256 + sub*64 + (il - 64*ph)
                    dst = rt[:].rearrange("r (s c) -> r s c", s=8)[
                        :, :, ph * 64 : (ph + 1) * 64
                    ]
                    src = accs_[0][:].rearrange("r (s c) -> r s c", s=8)
                    if last_b and ph == 1:
                        # ACT is idle after the final gelu; split the final
                        # evacuation so both halves go in parallel.
                        nc.vector.tensor_scalar(
                            dst[:, 0:4], src[:, 0:4], b2_sb, 0.0,
                            op0=ALU.add, op1=ALU.max,
                        )
                        nc.scalar.activation(
                            dst[:, 4:8], src[:, 4:8], AF.Relu, bias=b2_sb,
                            scale=1.0,
                        )
                    else:
                        nc.vector.tensor_scalar(
                            dst, src, b2_sb, 0.0, op0=ALU.add, op1=ALU.max
                        )
                    # mask + stage this il-phase now: the diagonal mask means
                    # values come back from the AllToAll already masked, and
                    # phase 0's 128KB stage runs mid-batch, off the tail
                    rt3 = rt[:].rearrange("r (s c) -> r s c", s=8)[
                        :, :, ph * 64 : (ph + 1) * 64
                    ]
                    m3 = masks_sb.rearrange("r (s c) -> r s c", s=8)[
                        :, :, ph * 64 : (ph + 1) * 64
                    ]
                    nc.vector.tensor_tensor(rt3, rt3, m3, op=ALU.mult)
                    st = nc.gpsimd if (last_b and ph == 1) else nc.sync
                    st.dma_start(
                        a2a_send[b][:, :, ph * 64 : (ph + 1) * 64].rearrange(
                            "s r c -> r s c"
                        ),
                        rt3,
                    )

                groups = []
                c = 0
                cyc = [("sA", 3), ("sB", 3), ("sC", 1)]
                gi = 0
                while c < NCHUNK:
                    tag, n = cyc[gi % 3]
                    gi += 1
                    n = min(n, NCHUNK - c)
                    groups.append((tag, list(range(c, c + n))))
                    c += n

                for gidx, (tag, chunks) in enumerate(groups):
                    if b == 0 and gidx == 1 and 1 not in at_sb:
                        emit_atT(1)
                    glen = 512 * len(chunks)
                    width = 512 if tag == "sC" else 1536
                    ps = ps_s.tile([128, width], F32, tag=tag)
                    # bias seeds: ps[:, c] = atT[row] (broadcast identity col)
                    for ci, C in enumerate(chunks):
                        row = C // 2
                        nc.tensor.matmul(
                            ps[:, ci * 512 : (ci + 1) * 512],
                            at_sb[b][:],
                            eye_sb[:, row : row + 1].broadcast_to((128, 512)),
                            start=True,
                            stop=False,
                        )
                    # S accumulate: ps[:, c] += W_row^T @ X[:, cols]
                    for ci, C in enumerate(chunks):
                        row, xcol = C // 2, (C % 2) * 512
                        if row not in wtiles:
                            wt = wp_pool.tile([D, H], F32R, tag="wi")
                            nc.vector.scalar_tensor_tensor(
                                wt[:],
                                wp_sb,
                                xc_sb[b][:, row : row + 1],
                                wb_sb,
                                op0=ALU.mult,
                                op1=ALU.add,
                            )
                            wtiles = {row: wt}  # keep only the latest
                        nc.tensor.matmul(
                            ps[:, ci * 512 : (ci + 1) * 512],
                            wtiles[row][:],
                            xtr[:, xcol : xcol + 512],
                            start=False,
                            stop=True,
                        )
                    gt = g_pool.tile([128, 1536], F16, tag="g")
                    nc.scalar.activation(
                        gt[:, 0:glen], ps[:, 0:glen], AF.Gelu, bias=0.0, scale=1.0
                    )
                    for ci, C in enumerate(chunks):
                        il = C // 2
                        phase = il // 64
                        if phase != cur_phase:
                            if accs is not None:
                                emit_evac(cur_phase, accs)
                            accs = [
                                ps_acc.tile(
                                    [128, 512],
                                    F32,
                                    tag="acc",
                                    name=f"acc_{rep}_{b}_{phase}",
                                )
                            ]
                            cur_phase = phase
                        q = C % 2
                        for s in range(4):
                            sub = s  # jt % 4
                            col = q * 256 + sub * 64 + (il - 64 * phase)
                            nc.tensor.matmul(
                                accs[0][:, col : col + 1],
                                gt[:, (ci * 4 + s) * 128 : (ci * 4 + s + 1) * 128],
                                w2_sb,
                                start=True,
                                stop=True,
                            )
                emit_evac(cur_phase, accs)

                # mask + stage: zero this core's diagonal block BEFORE
                # staging so the values come back from the AllToAll already
                # masked; then one DMA to the A2A send buffer.
                rt = rt_tiles[b]

                # all-to-all this batch's transposed-slab blocks
                if not skip_collective:
                    nc.gpsimd.collective_compute(
                        "AllToAll",
                        ALU.bypass,
                        replica_groups=[list(range(NCORES))],
                        ins=[a2a_send[b].opt()],
                        outs=[a2a_recv[b].opt()],
                    )


                # symmetrize: out[b, d-block, :] = own + recv^T (the diag
                # mask was already applied to rt pre-exchange).  One merged
                # output store; HWDGE issues only ~1 DMA per 1.26us, so the
                # tail must use few DMAs.
                ob_all = fin_pool.tile([128, 8 * SLAB], F16, name=f"ob_{b}")
                if last_b:
                    # final batch: one merged recv load, transposed on the
                    # (now idle) PE against the identity, adds on DVE.  Each
                    # transpose gets its own PSUM bank -- a shared bank would
                    # serialize transpose_d+1 behind add_d (bank-granular
                    # accumulation groups).
                    rv = fin_pool.tile([128, 8 * SLAB], F16, name=f"rv_{b}")
                    nc.scalar.dma_start(
                        rv[:].rearrange("r (d c) -> r d c", d=8),
                        a2a_recv[b][:].rearrange("d r c -> r d c"),
                    )
                    tb_sa = ps_s.tile([128, 3072], F16, tag="sA")
                    tb_sb = ps_s.tile([128, 3072], F16, tag="sB")
                    tb_sc = ps_s.tile([128, 1024], F16, tag="sC")
                    tb_ac = ps_acc.tile([128, 512], F32, tag="acc")
                    banks = (
                        [(tb_sa, 1024 * k) for k in range(3)]
                        + [(tb_sb, 1024 * k) for k in range(3)]
                        + [(tb_sc, 0), (tb_ac, 0)]
                    )
                    for d in range(NCORES):
                        sl = slice(d * 128, (d + 1) * 128)
                        tbt, off = banks[d]
                        dst = tbt[:, off : off + 256].bitcast(F16)[:, 0:128] if (
                            tbt is tb_ac
                        ) else tbt[:, off : off + 128]
                        nc.tensor.transpose(dst, rv[:, sl], eye_sb)
                        nc.vector.tensor_tensor(
                            ob_all[:, sl], dst, rt[:, sl], op=ALU.add
                        )
                        if d == 3:
                            nc.scalar.dma_start(
                                out_t[b, 0:512].rearrange("(d p) f -> p d f", d=4),
                                ob_all[:, 0:512].rearrange(
                                    "p (d f) -> p d f", d=4
                                ),
                            )
                    nc.sync.dma_start(
                        out_t[b, 512:1024].rearrange("(d p) f -> p d f", d=4),
                        ob_all[:, 512:1024].rearrange("p (d f) -> p d f", d=4),
                    )
                else:
                    # recv blocks transposed in-flight by the DMA xbar
                    for d in range(NCORES):
                        rbt = fin_pool.tile([128, 128], F16, tag="rbt")
                        nc.sync.dma_start_transpose(rbt[:], a2a_recv[b][d])
                        nc.gpsimd.tensor_tensor(
                            ob_all[:, d * 128 : (d + 1) * 128],
                            rbt[:],
                            rt[:, d * 128 : (d + 1) * 128],
                            op=ALU.add,
                        )
                if not last_b:
                    nc.gpsimd.dma_start(
                        out_t[b].rearrange("(d p) f -> p d f", d=8),
                        ob_all[:].rearrange("p (d f) -> p d f", d=8),
                    )

    nc.compile()
    return nc


_NC_CACHE = {}


def _get_nc():
    if "nc" not in _NC_CACHE:
        _NC_CACHE["nc"] = build_nc()
    return _NC_CACHE["nc"]


def make_in_maps(X, W1, b1, W2, b2):
    X = np.ascontiguousarray(X, dtype=np.float32)
    W1 = np.asarray(W1, dtype=np.float32)
    b1 = np.asarray(b1, dtype=np.float32)
    W2 = np.asarray(W2, dtype=np.float32)
    b2 = np.asarray(b2, dtype=np.float32)

    Wi, Wj, Wd, Wp = W1[0:128], W1[128:256], W1[256:384], W1[384:512]
    wa = Wi + Wd
    wb = Wj - Wd
    wp = Wp
    w2h = (0.5 * W2).astype(np.float16).reshape(H, 1)
    b1r = np.tile(b1.reshape(1, H), (128, 1))
    b2c = np.full((128, 1), 0.5 * float(b2[0]), dtype=np.float32)
    eye = np.eye(128, dtype=np.float16)
    xt = np.ascontiguousarray(X.transpose(0, 2, 1))  # (B, D, L)

    in_maps = []
    for c in range(NCORES):
        masks = np.ones((128, NCORES * 128), dtype=np.float16)
        masks[:, c * 128 : (c + 1) * 128] = (
            1.0 - np.eye(128)
        ).astype(np.float16)
        xc = np.ascontiguousarray(xt[:, :, c * SLAB : (c + 1) * SLAB])
        cst = np.ascontiguousarray(
            np.concatenate([xc[0], wa, wp, wb, b1r, b2c], axis=1)
        ).astype(np.float32)
        pad = np.zeros((128, 1), np.float16)
        cst16 = np.ascontiguousarray(
            np.concatenate([eye, w2h, pad, masks], axis=1)
        ).astype(np.float16)
        in_maps.append({"xt": xt, "xc": xc, "cst": cst, "cst16": cst16})
    return in_maps


def assemble(results):
    full = np.empty((B, L, L), dtype=np.float32)
    for c in range(NCORES):
        o = results[c]["out"]  # (B, L, SLAB) fp16: out[b, j, i_local]
        full[:, c * SLAB : (c + 1) * SLAB, :] = o.transpose(0, 2, 1).astype(
            np.float32
        )
    return full


def kernel(X, W1, b1, W2, b2, _trace=False):
    nc = _get_nc()
    in_maps = make_in_maps(X, W1, b1, W2, b2)
    res = run_bass_kernel_spmd(
        nc, in_maps, core_ids=list(range(NCORES)), trace=_trace
    )
    out = assemble(res.results)
    if _trace:
        return out, res
    return out


if __name__ == "__main__":
    nc = build_nc()
    print("compiled ok")
